# revision 10
# baseline (speedup 1.0000x reference)
"""Trainium2 Bass kernel for StyleGAN2-style modulated conv2d (ModConv2D).

Reference computation (per sample b):
    w      = kernel * (style[b] + 1)                 # modulate [3,3,Cin,Cout]
    w      = w / sqrt(sum(w^2, (kh,kw,Cin)) + 1e-8)  # demodulate per Cout
    y[b]   = conv2d_same(x[b], w)

Sharding: data-parallel over batch — 16 samples across 8 NeuronCores,
2 samples per core; the base kernel is replicated.

Device algorithm per core (2 samples):
  - conv as 9-tap accumulated matmuls: psum[cout,pix] += w[t,cin,cout]^T @
    xT[cin, pix+off].  x is held channel-major FLAT ([cin, cc, 64+4096+80]
    fp16) with zero guard rows; horizontal (dx=+-1) taps use column-split
    matmuls (N=504, strided psum out) so row wrap never leaks.
  - x ingest transposes run on the PE (transpose-matmul + batched DVE
    eviction); output transposes ride the DMA xbar split across both HWDGE
    rings.  Weights are modulated on-chip per-tap so the first conv group
    unblocks as the per-tap kernel DMAs land.
  - demod factor d[cout] = rsqrt(sum_cin s^2 * K2 + 1e-8) in fp32 on device
    (K2 = sum_t kernel^2 once per core) computed COLUMN-WISE via N=1
    matmuls (k2^T @ s2col), applied per-partition on psum eviction
    (oc0 -> scalar ACT, oc1 -> DVE tensor_scalar; balances the rings).
    Output staged fp16, cast back to fp32 by the store DMA (SWDGE).
  - Scheduling: identity is built before the x-load issues on gpsimd; x
    block (0,0) loads per-128-pixel quarter so the first PE transposes
    start ~1.5us in; the demod matmuls (which wait on K2, ~9us) are
    emitted AFTER conv tiles 0-1's matmuls with those tiles' evictions
    deferred, so the PE ramps straight into the conv; sample 1's
    modulation/demod are emitted after conv tile 2 so they can never
    block sample 0's startup-critical work.  Final tile transposes on
    the then-idle PE to shorten the tail.
"""

import numpy as np

B, H, W, CIN, COUT, KH, KW = 16, 64, 64, 256, 256, 3, 3
NCORES = 8
BPC = B // NCORES  # samples per core
T = KH * KW  # 9 taps
HWPIX = H * W  # 4096
PAD0 = 64  # zero pixels before the image
XLEN = PAD0 + HWPIX + 80  # 4240: multiple of 16 so xbar dest strides stay 32B-aligned

# tap order: dx=0 taps first so the first matmul of each psum group writes all
# 512 columns with start=True
TAP_ORDER = [1, 4, 7, 0, 3, 6, 2, 5, 8]

_CACHE = {}
LAST_EXEC_NS = None
LAST_MEAN_EXEC_NS = None


def _build_nc():
    from contextlib import ExitStack

    import concourse.bacc as bacc
    import concourse.bass as bass
    import concourse.mybir as mybir
    import concourse.tile as tile
    from concourse.masks import make_identity

    f32 = mybir.dt.float32
    f16 = mybir.dt.float16  # fp16: same 1 cyc/row PE rate as bf16, 4x finer mantissa
    AF = mybir.ActivationFunctionType

    nc = bacc.Bacc("TRN2", target_bir_lowering=False, debug=False)

    x_d = nc.dram_tensor("x", [BPC, H, W, CIN], f32, kind="ExternalInput")
    s_d = nc.dram_tensor("style", [BPC, CIN], f32, kind="ExternalInput")
    k_d = nc.dram_tensor("kernel", [KH, KW, CIN, COUT], f32, kind="ExternalInput")
    y_d = nc.dram_tensor("y", [BPC, H, W, COUT], f32, kind="ExternalOutput")

    XB = H * W * CIN  # x/y sample stride (elements)
    KKW = CIN * COUT  # kernel tap stride

    def x_sub_ap(b, t8, s, ns):
        # [128*ns pix, 256 cin] starting at pixel (t8*4+s)*128
        off = b * XB + (t8 * 4 + s) * 128 * CIN
        return bass.AP(x_d, off, [[CIN, 128], [128 * CIN, ns], [1, CIN]])

    def y_blk_ap(b, t8):
        off = b * XB + t8 * 4 * 128 * COUT
        return bass.AP(y_d, off, [[COUT, 128], [128 * COUT, 4], [1, COUT]])

    def k_tap_ap(cc, t):
        # [128 cin, 256 cout] for one tap
        return bass.AP(k_d, t * KKW + cc * 128 * COUT, [[COUT, 128], [1, COUT]])

    with tile.TileContext(nc) as tc, ExitStack() as ctx:
        singles = ctx.enter_context(tc.tile_pool(name="singles", bufs=1))
        tmp_pool = ctx.enter_context(tc.tile_pool(name="tmp", bufs=1))
        wpool = ctx.enter_context(tc.tile_pool(name="wpool", bufs=2))
        dpool = ctx.enter_context(tc.tile_pool(name="dpool", bufs=10))
        srow_pool = ctx.enter_context(tc.tile_pool(name="srow", bufs=4))
        xpool = ctx.enter_context(tc.tile_pool(name="xpool", bufs=2))
        xtpool = ctx.enter_context(tc.tile_pool(name="xt", bufs=2 * 8))
        ospool = ctx.enter_context(tc.tile_pool(name="osb", bufs=6))
        obpool = ctx.enter_context(tc.tile_pool(name="ob", bufs=4))
        pconv = ctx.enter_context(tc.tile_pool(name="pconv", bufs=5, space="PSUM"))
        pxt = ctx.enter_context(tc.tile_pool(name="pxt", bufs=2, space="PSUM"))
        psmall = ctx.enter_context(tc.tile_pool(name="psmall", bufs=1, space="PSUM"))

        # style rows (tiny, first on the sync ring)
        srows = []
        for b in range(BPC):
            srow = srow_pool.tile([1, CIN], f32, tag=f"srow{b}")
            nc.sync.dma_start(out=srow, in_=s_d.ap()[b : b + 1, :])
            srows.append(srow)
        xts = [[None] * 8 for _ in range(BPC)]

        def alloc_xtmp(b, t8):
            xtmp = xtpool.tile([128, 4, CIN], f16, tag="xtmp", name=f"xtmp_{b}_{t8}")
            xts[b][t8] = xtmp
            return xtmp

        # blocks (0,0) and (0,1) ride the HWDGE rings as fp32, AHEAD of the
        # taps (HWDGE DMAs dispatch ~6us before any compute engine wakes, but
        # cannot cast) and are DVE-cast to fp16 the moment the DVE comes up —
        # this beats the gpsimd-issued cast path to first-transpose and frees
        # the SWDGE queue for the later blocks
        xstage = []
        for t8 in range(2):
            xs = tmp_pool.tile([128, 4, CIN], f32, tag=f"xstage{t8}")
            eng = nc.sync if t8 == 0 else nc.scalar
            eng.dma_start(out=xs, in_=x_sub_ap(0, t8, 0, 4))
            xstage.append(xs)

        # per-tap kernel loads in conv tap order (the modulated weights gate
        # the conv ramp), alternating HWDGE rings
        kbase = singles.tile([128, 2, T, COUT], f32)
        for ti, t in enumerate(TAP_ORDER):
            for cc in range(2):
                eng = nc.sync if (ti * 2 + cc) % 2 == 0 else nc.scalar
                eng.dma_start(out=kbase[:, cc, t], in_=k_tap_ap(cc, t))

        ident_b = singles.tile([128, 128], f16)
        make_identity(nc, ident_b)
        for t8 in range(2):
            nc.vector.tensor_copy(out=alloc_xtmp(0, t8), in_=xstage[t8])
        # remaining x loads (cast fp32->fp16, SWDGE), whole blocks: each
        # gpsimd issue costs ~870ns, so fewer issues beat finer granularity
        for t8 in range(2, 8):
            nc.gpsimd.dma_start(out=alloc_xtmp(0, t8), in_=x_sub_ap(0, t8, 0, 4))
        for t8 in range(8):
            nc.gpsimd.dma_start(out=alloc_xtmp(1, t8), in_=x_sub_ap(1, t8, 0, 4))

        ones1 = singles.tile([1, 1], f32)
        nc.vector.memset(ones1, 1.0)
        eps_sb = singles.tile([128, 1], f32)
        nc.vector.memset(eps_sb, 1e-8)

        # xflat guard memsets early on DVE (no input deps; they gate conv
        # tile 0's dy=-1 taps)
        xflats = []
        for b in range(BPC):
            xflat = xpool.tile([128, 2, XLEN], f16, tag="xflat", name=f"xflat{b}")
            nc.vector.memset(xflat[:, :, 0:PAD0], 0.0)
            nc.vector.memset(xflat[:, :, PAD0 + HWPIX : XLEN], 0.0)
            xflats.append(xflat)

        wbs, dsbs, s2cs = {}, {}, {}

        # one bank-shared psum tile, column slots for all tiny matmuls:
        # independent columns -> no WAR serialization between the chains
        pcol8 = psmall.tile([128, 8], f32, tag="pcol8")

        def modulation(b):
            srow1 = srow_pool.tile([1, CIN], f32, tag=f"srow1_{b}")
            nc.vector.tensor_scalar_add(srow1, srows[b], 1.0)

            smod = dpool.tile([128, 2], f32, tag=f"smod{b}")  # (style+1) col-major
            for cc in range(2):
                pcol = pcol8[:, 4 * b + cc : 4 * b + cc + 1]
                nc.tensor.matmul(
                    pcol, srow1[:, cc * 128 : (cc + 1) * 128], ones1, start=True, stop=True
                )
                nc.vector.tensor_copy(out=smod[:, cc : cc + 1], in_=pcol)
            s2c = dpool.tile([128, 2], f32, tag=f"s2c{b}")
            nc.vector.tensor_mul(s2c, smod, smod)
            s2cs[b] = s2c

            # wb[cin, cc, t, cout] = kernel * (s+1), cast fp16, on ACT, per
            # tap in conv order so the first conv group unblocks early
            wb = wpool.tile([128, 2, T, COUT], f16, tag="wb")
            for t in TAP_ORDER:
                for cc in range(2):
                    nc.scalar.activation(
                        wb[:, cc, t], kbase[:, cc, t], AF.Copy,
                        scale=smod[:, cc : cc + 1],
                    )
            wbs[b] = wb

        def demod(b):
            # d[cout] = rsqrt(sum_cin s2*K2 + 1e-8) as a column, via two N=1
            # matmuls per cout half (emitted in the PE's startup shadow)
            s2c = s2cs[b]
            sqc = dpool.tile([128, 2], f32, tag=f"sqc{b}")
            for oc in range(2):
                pcol = pcol8[:, 4 * b + 2 + oc : 4 * b + 3 + oc]
                for cc in range(2):
                    nc.tensor.matmul(
                        pcol,
                        k2[:, cc, oc * 128 : (oc + 1) * 128],
                        s2c[:, cc : cc + 1],
                        start=(cc == 0),
                        stop=(cc == 1),
                    )
                nc.scalar.activation(sqc[:, oc : oc + 1], pcol, AF.Sqrt, bias=eps_sb)
            d_sb = dpool.tile([128, 2], f32, tag=f"d{b}")
            nc.vector.reciprocal(d_sb, sqc)
            dsbs[b] = d_sb

        modulation(0)
        if BPC > 1:
            # sample 1's modulation also upfront: its DVE ops land before k2
            # in the DVE queue (so never behind the eviction pipeline) and
            # its tiny PE matmuls fit the x-bound startup gaps
            modulation(1)

        def transpose_block(b, t8):
            # 8 PE transposes + 2 batched DVE evictions per xtmp
            xtmp = xts[b][t8]
            xflat = xflats[b]
            for cc in range(2):
                pxt_t = pxt.tile([128, 4, 128], f16, tag="pxt")
                for s in range(4):
                    nc.tensor.transpose(
                        pxt_t[:, s, :],
                        xtmp[:, s, cc * 128 : (cc + 1) * 128],
                        ident_b,
                    )
                nc.vector.tensor_copy(
                    out=xflat[:, cc, PAD0 + 512 * t8 : PAD0 + 512 * (t8 + 1)],
                    in_=pxt_t,
                )

        def conv_matmuls_oc(b, t8, oc):
            # accumulate one cout-half psum for output pixels t8*512..+511
            wb = wbs[b]
            xflat = xflats[b]
            p0 = t8 * 512
            ps = pconv.tile([128, 512], f32, tag="pconv")
            ps_r = ps.rearrange("p (r w) -> p r w", w=64)
            i = 0
            for t in TAP_ORDER:
                dy, dx = t // 3 - 1, t % 3 - 1
                base = PAD0 + p0 + 64 * dy
                for cc in range(2):
                    lhsT = wb[:, cc, t, oc * 128 : (oc + 1) * 128]
                    xf = xflat[:, cc]
                    if dx == 0:
                        rhs = xf[:, base : base + 512]
                        out_ap = ps
                    elif dx == -1:
                        rhs = xf[:, base : base + 512].rearrange(
                            "p (r w) -> p r w", w=64
                        )[:, :, 0:63]
                        out_ap = ps_r[:, :, 1:64]
                    else:  # dx == +1
                        rhs = xf[:, base + 1 : base + 513].rearrange(
                            "p (r w) -> p r w", w=64
                        )[:, :, 0:63]
                        out_ap = ps_r[:, :, 0:63]
                    nc.tensor.matmul(out_ap, lhsT, rhs, start=(i == 0), stop=(i == 17))
                    i += 1
            return ps

        def conv_matmuls(b, t8):
            return [conv_matmuls_oc(b, t8, 0), conv_matmuls_oc(b, t8, 1)]

        def conv_evict_oc(b, t8, oc, ps, ob, last=False):
            d_sb = dsbs[b]
            o_sb = ospool.tile([128, 512], f16, tag="osb")
            # demod scale + fp32->fp16; oc0 on the ACT ring, oc1 on the
            # DVE so neither ring carries both evictions.  The final tile
            # evicts on the DVE only: the scalar ring may be mid-way through
            # a 1.2us xbar-transpose issue right when the tail starts.
            if oc == 0 and not last:
                nc.scalar.activation(o_sb, ps, AF.Copy, scale=d_sb[:, oc : oc + 1])
            else:
                nc.vector.tensor_scalar_mul(o_sb, ps, d_sb[:, oc : oc + 1])
            if last:
                # final tile: PE transpose (reusing the ingest psum pool,
                # idle by now) — shorter tail than xbar+DGE — and ship
                # each cout half as soon as it is ready
                pot_t = pxt.tile([128, 4, 128], f16, tag="pxt")
                for s in range(4):
                    nc.tensor.transpose(
                        pot_t[:, s, :], o_sb[:, s * 128 : (s + 1) * 128], ident_b
                    )
                nc.vector.tensor_copy(out=ob[:, :, oc * 128 : (oc + 1) * 128], in_=pot_t)
                yb = y_blk_ap(b, t8)
                half = bass.AP(
                    yb.tensor,
                    yb.offset + oc * 128,
                    [[COUT, 128], [128 * COUT, 4], [1, 128]],
                )
                nc.gpsimd.dma_start(out=half, in_=ob[:, :, oc * 128 : (oc + 1) * 128])
            else:
                # output transpose on the DMA xbar, split across rings
                eng = nc.sync if oc == 0 else nc.scalar
                eng.dma_start_transpose(out=ob[:, :, oc * 128 : (oc + 1) * 128], in_=o_sb)

        def conv_evict(b, t8, pss):
            ob = obpool.tile([128, 4, COUT], f16, tag="ob")
            for oc in range(2):
                conv_evict_oc(b, t8, oc, pss[oc], ob)
            nc.gpsimd.dma_start(out=y_blk_ap(b, t8), in_=ob)

        # K2[cin, cout] = sum_t kernel^2 (once per core).  Squared taps are
        # staged fp16 (2x DVE rate on the reduce read); accumulation and k2
        # stay fp32.  Emitted after the startup-critical DVE work; the demod
        # matmuls that consume it are deferred past conv tiles 0-1 so the PE
        # never waits for it.
        k2 = singles.tile([128, 2, COUT], f32)

        def compute_k2():
            for cc in range(2):
                k2tmp = tmp_pool.tile([128, T, COUT], f16, tag="k2tmp")
                nc.vector.tensor_mul(k2tmp, kbase[:, cc], kbase[:, cc])
                nc.vector.reduce_sum(
                    out=k2[:, cc],
                    in_=k2tmp.rearrange("p t c -> p c t"),
                    axis=mybir.AxisListType.X,
                )

        items = [(b, t8) for b in range(BPC) for t8 in range(8)]
        PF = 2  # transpose prefetch distance ahead of conv

        transpose_block(*items[0])
        transpose_block(*items[1])
        ps0 = conv_matmuls(*items[0])
        transpose_block(*items[2])
        compute_k2()
        ps1 = conv_matmuls(*items[1])
        transpose_block(*items[3])
        demod(0)
        if BPC > 1:
            demod(1)
        conv_evict(*items[0], ps0)
        conv_evict(*items[1], ps1)
        for i, (b, t8) in enumerate(items):
            if i < 2:
                continue
            if i + PF < len(items):
                transpose_block(*items[i + PF])
            if i == len(items) - 1:
                # last tile per-oc: oc0's evict/transpose/store overlaps
                # oc1's matmuls, shortening the tail
                ob = obpool.tile([128, 4, COUT], f16, tag="ob")
                for oc in range(2):
                    ps = conv_matmuls_oc(b, t8, oc)
                    conv_evict_oc(b, t8, oc, ps, ob, last=True)
            else:
                conv_evict(b, t8, conv_matmuls(b, t8))

    nc.compile()
    return nc


def _get_nc():
    if "nc" not in _CACHE:
        _CACHE["nc"] = _build_nc()
    return _CACHE["nc"]


def kernel(x, style, kernel, _trace=False):
    global LAST_EXEC_NS, LAST_MEAN_EXEC_NS
    from concourse.bass_utils import run_bass_kernel_spmd

    x = np.ascontiguousarray(x, dtype=np.float32)
    style = np.ascontiguousarray(style, dtype=np.float32)
    kern = np.ascontiguousarray(kernel, dtype=np.float32)

    nc = _get_nc()
    in_maps = [
        {
            "x": x[i * BPC : (i + 1) * BPC],
            "style": style[i * BPC : (i + 1) * BPC],
            "kernel": kern,
        }
        for i in range(NCORES)
    ]
    res = run_bass_kernel_spmd(nc, in_maps, core_ids=list(range(NCORES)), trace=_trace)
    LAST_EXEC_NS = res.exec_time_ns
    LAST_MEAN_EXEC_NS = res.mean_exec_time_ns
    return np.concatenate([res.results[i]["y"] for i in range(NCORES)], axis=0)


# revision 14
# speedup vs baseline: 1.0247x; 1.0247x over previous
"""Trainium2 Bass kernel for StyleGAN2-style modulated conv2d (ModConv2D).

Reference computation (per sample b):
    w      = kernel * (style[b] + 1)                 # modulate [3,3,Cin,Cout]
    w      = w / sqrt(sum(w^2, (kh,kw,Cin)) + 1e-8)  # demodulate per Cout
    y[b]   = conv2d_same(x[b], w)

Sharding: data-parallel over batch — 16 samples across 8 NeuronCores,
2 samples per core; the base kernel is replicated.

Device algorithm per core (2 samples):
  - conv as 9-tap accumulated matmuls: psum[cout,pix] += w[t,cin,cout]^T @
    xT[cin, pix+off].  x is held channel-major FLAT ([cin, cc, 64+4096+80]
    fp16) with zero guard rows; horizontal (dx=+-1) taps use column-split
    matmuls (N=504, strided psum out) so row wrap never leaks.
  - x ingest transposes run on the PE (transpose-matmul + batched DVE
    eviction); output transposes ride the DMA xbar split across both HWDGE
    rings.  Weights are modulated on-chip per-tap so the first conv group
    unblocks as the per-tap kernel DMAs land.
  - demod factor d[cout] = rsqrt(sum_cin s^2 * K2 + 1e-8) in fp32 on device
    (K2 = sum_t kernel^2 once per core) computed COLUMN-WISE via N=1
    matmuls (k2^T @ s2col), applied per-partition on psum eviction
    (oc0 -> scalar ACT, oc1 -> DVE tensor_scalar; balances the rings).
    Output staged fp16, cast back to fp32 by the store DMA (SWDGE).
  - Scheduling: identity is built before the x-load issues on gpsimd; x
    block (0,0) loads per-128-pixel quarter so the first PE transposes
    start ~1.5us in; the demod matmuls (which wait on K2, ~9us) are
    emitted AFTER conv tiles 0-1's matmuls with those tiles' evictions
    deferred, so the PE ramps straight into the conv; sample 1's
    modulation/demod are emitted after conv tile 2 so they can never
    block sample 0's startup-critical work.  Final tile transposes on
    the then-idle PE to shorten the tail.
"""

import numpy as np

B, H, W, CIN, COUT, KH, KW = 16, 64, 64, 256, 256, 3, 3
NCORES = 8
BPC = B // NCORES  # samples per core
T = KH * KW  # 9 taps
HWPIX = H * W  # 4096
PAD0 = 64  # zero pixels before the image
XLEN = PAD0 + HWPIX + 80  # 4240: multiple of 16 so xbar dest strides stay 32B-aligned

# tap order: dx=0 taps first so the first matmul of each psum group writes all
# 512 columns with start=True
TAP_ORDER = [1, 4, 7, 0, 3, 6, 2, 5, 8]

_CACHE = {}
LAST_EXEC_NS = None
LAST_MEAN_EXEC_NS = None


def _build_nc():
    from contextlib import ExitStack

    import concourse.bacc as bacc
    import concourse.bass as bass
    import concourse.mybir as mybir
    import concourse.tile as tile
    from concourse.masks import make_identity

    f32 = mybir.dt.float32
    f16 = mybir.dt.float16  # fp16: same 1 cyc/row PE rate as bf16, 4x finer mantissa
    AF = mybir.ActivationFunctionType

    nc = bacc.Bacc("TRN2", target_bir_lowering=False, debug=False)

    x_d = nc.dram_tensor("x", [BPC, H, W, CIN], f32, kind="ExternalInput")
    s_d = nc.dram_tensor("style", [BPC, CIN], f32, kind="ExternalInput")
    k_d = nc.dram_tensor("kernel", [KH, KW, CIN, COUT], f32, kind="ExternalInput")
    y_d = nc.dram_tensor("y", [BPC, H, W, COUT], f32, kind="ExternalOutput")

    XB = H * W * CIN  # x/y sample stride (elements)
    KKW = CIN * COUT  # kernel tap stride

    def x_sub_ap(b, t8, s, ns):
        # [128*ns pix, 256 cin] starting at pixel (t8*4+s)*128
        off = b * XB + (t8 * 4 + s) * 128 * CIN
        return bass.AP(x_d, off, [[CIN, 128], [128 * CIN, ns], [1, CIN]])

    def y_blk_ap(b, t8):
        off = b * XB + t8 * 4 * 128 * COUT
        return bass.AP(y_d, off, [[COUT, 128], [128 * COUT, 4], [1, COUT]])

    def k_tap_ap(cc, t):
        # [128 cin, 256 cout] for one tap
        return bass.AP(k_d, t * KKW + cc * 128 * COUT, [[COUT, 128], [1, COUT]])

    with tile.TileContext(nc) as tc, ExitStack() as ctx:
        singles = ctx.enter_context(tc.tile_pool(name="singles", bufs=1))
        tmp_pool = ctx.enter_context(tc.tile_pool(name="tmp", bufs=1))
        wpool = ctx.enter_context(tc.tile_pool(name="wpool", bufs=2))
        dpool = ctx.enter_context(tc.tile_pool(name="dpool", bufs=10))
        srow_pool = ctx.enter_context(tc.tile_pool(name="srow", bufs=4))
        xpool = ctx.enter_context(tc.tile_pool(name="xpool", bufs=2))
        xtpool = ctx.enter_context(tc.tile_pool(name="xt", bufs=2 * 8))
        ospool = ctx.enter_context(tc.tile_pool(name="osb", bufs=6))
        obpool = ctx.enter_context(tc.tile_pool(name="ob", bufs=4))
        pconv = ctx.enter_context(tc.tile_pool(name="pconv", bufs=5, space="PSUM"))
        pxt = ctx.enter_context(tc.tile_pool(name="pxt", bufs=2, space="PSUM"))
        psmall = ctx.enter_context(tc.tile_pool(name="psmall", bufs=1, space="PSUM"))

        # style rows (tiny, first on the sync ring)
        srows = []
        for b in range(BPC):
            srow = srow_pool.tile([1, CIN], f32, tag=f"srow{b}")
            nc.sync.dma_start(out=srow, in_=s_d.ap()[b : b + 1, :])
            srows.append(srow)
        xts = [[None] * 8 for _ in range(BPC)]

        def alloc_xtmp(b, t8):
            xtmp = xtpool.tile([128, 4, CIN], f16, tag="xtmp", name=f"xtmp_{b}_{t8}")
            xts[b][t8] = xtmp
            return xtmp

        # per-tap kernel loads in conv tap order (the modulated weights gate
        # the conv ramp), alternating HWDGE rings
        kbase = singles.tile([128, 2, T, COUT], f32)
        for ti, t in enumerate(TAP_ORDER):
            for cc in range(2):
                eng = nc.sync if (ti * 2 + cc) % 2 == 0 else nc.scalar
                eng.dma_start(out=kbase[:, cc, t], in_=k_tap_ap(cc, t))

        # x loads (cast fp32->fp16, SWDGE), whole blocks: each gpsimd issue
        # costs ~870ns of engine time, so fewer issues beat finer granularity
        nc.gpsimd.dma_start(out=alloc_xtmp(0, 0), in_=x_sub_ap(0, 0, 0, 4))
        nc.gpsimd.dma_start(out=alloc_xtmp(0, 1), in_=x_sub_ap(0, 1, 0, 4))
        ident_b = singles.tile([128, 128], f16)
        make_identity(nc, ident_b)
        for t8 in range(2, 8):
            nc.gpsimd.dma_start(out=alloc_xtmp(0, t8), in_=x_sub_ap(0, t8, 0, 4))
        for t8 in range(8):
            nc.gpsimd.dma_start(out=alloc_xtmp(1, t8), in_=x_sub_ap(1, t8, 0, 4))

        ones1 = singles.tile([1, 1], f32)
        nc.vector.memset(ones1, 1.0)
        eps_sb = singles.tile([128, 1], f32)
        nc.vector.memset(eps_sb, 1e-8)

        # xflat guard memsets early on DVE (no input deps; they gate conv
        # tile 0's dy=-1 taps)
        xflats = []
        for b in range(BPC):
            xflat = xpool.tile([128, 2, XLEN], f16, tag="xflat", name=f"xflat{b}")
            nc.vector.memset(xflat[:, :, 0:PAD0], 0.0)
            nc.vector.memset(xflat[:, :, PAD0 + HWPIX : XLEN], 0.0)
            xflats.append(xflat)

        wbs, dsbs, s2cs = {}, {}, {}

        # one bank-shared psum tile, column slots for all tiny matmuls:
        # independent columns -> no WAR serialization between the chains
        pcol8 = psmall.tile([128, 8], f32, tag="pcol8")

        def modulation(b):
            srow1 = srow_pool.tile([1, CIN], f32, tag=f"srow1_{b}")
            nc.vector.tensor_scalar_add(srow1, srows[b], 1.0)

            smod = dpool.tile([128, 2], f32, tag=f"smod{b}")  # (style+1) col-major
            for cc in range(2):
                pcol = pcol8[:, 4 * b + cc : 4 * b + cc + 1]
                nc.tensor.matmul(
                    pcol, srow1[:, cc * 128 : (cc + 1) * 128], ones1, start=True, stop=True
                )
                nc.vector.tensor_copy(out=smod[:, cc : cc + 1], in_=pcol)
            s2c = dpool.tile([128, 2], f32, tag=f"s2c{b}")
            nc.vector.tensor_mul(s2c, smod, smod)
            s2cs[b] = s2c

            # wb[cin, cc, t, cout] = kernel * (s+1), cast fp16, on ACT, per
            # tap in conv order so the first conv group unblocks early
            wb = wpool.tile([128, 2, T, COUT], f16, tag="wb")
            for t in TAP_ORDER:
                for cc in range(2):
                    nc.scalar.activation(
                        wb[:, cc, t], kbase[:, cc, t], AF.Copy,
                        scale=smod[:, cc : cc + 1],
                    )
            wbs[b] = wb

        def demod(b):
            # d[cout] = rsqrt(sum_cin s2*K2 + 1e-8) as a column, via two N=1
            # matmuls per cout half (emitted in the PE's startup shadow)
            s2c = s2cs[b]
            sqc = dpool.tile([128, 2], f32, tag=f"sqc{b}")
            for oc in range(2):
                pcol = pcol8[:, 4 * b + 2 + oc : 4 * b + 3 + oc]
                for cc in range(2):
                    nc.tensor.matmul(
                        pcol,
                        k2[:, cc, oc * 128 : (oc + 1) * 128],
                        s2c[:, cc : cc + 1],
                        start=(cc == 0),
                        stop=(cc == 1),
                    )
                nc.scalar.activation(sqc[:, oc : oc + 1], pcol, AF.Sqrt, bias=eps_sb)
            d_sb = dpool.tile([128, 2], f32, tag=f"d{b}")
            nc.vector.reciprocal(d_sb, sqc)
            dsbs[b] = d_sb

        modulation(0)
        if BPC > 1:
            # sample 1's modulation also upfront: its DVE ops land before k2
            # in the DVE queue (so never behind the eviction pipeline) and
            # its tiny PE matmuls fit the x-bound startup gaps
            modulation(1)

        def transpose_block(b, t8):
            # 8 PE transposes + 2 batched DVE evictions per xtmp
            xtmp = xts[b][t8]
            xflat = xflats[b]
            for cc in range(2):
                pxt_t = pxt.tile([128, 4, 128], f16, tag="pxt")
                for s in range(4):
                    nc.tensor.transpose(
                        pxt_t[:, s, :],
                        xtmp[:, s, cc * 128 : (cc + 1) * 128],
                        ident_b,
                    )
                nc.vector.tensor_copy(
                    out=xflat[:, cc, PAD0 + 512 * t8 : PAD0 + 512 * (t8 + 1)],
                    in_=pxt_t,
                )

        def mm_taps(b, t8, oc, ps, taps, first, final):
            # accumulate a subset of taps into one cout-half psum
            wb = wbs[b]
            xflat = xflats[b]
            p0 = t8 * 512
            ps_r = ps.rearrange("p (r w) -> p r w", w=64)
            n = len(taps) * 2
            i = 0
            for t in taps:
                dy, dx = t // 3 - 1, t % 3 - 1
                base = PAD0 + p0 + 64 * dy
                for cc in range(2):
                    lhsT = wb[:, cc, t, oc * 128 : (oc + 1) * 128]
                    xf = xflat[:, cc]
                    if dx == 0:
                        rhs = xf[:, base : base + 512]
                        out_ap = ps
                    elif dx == -1:
                        rhs = xf[:, base : base + 512].rearrange(
                            "p (r w) -> p r w", w=64
                        )[:, :, 0:63]
                        out_ap = ps_r[:, :, 1:64]
                    else:  # dx == +1
                        rhs = xf[:, base + 1 : base + 513].rearrange(
                            "p (r w) -> p r w", w=64
                        )[:, :, 0:63]
                        out_ap = ps_r[:, :, 0:63]
                    nc.tensor.matmul(
                        out_ap, lhsT, rhs,
                        start=(first and i == 0),
                        stop=(final and i == n - 1),
                    )
                    i += 1

        def conv_matmuls_oc(b, t8, oc):
            ps = pconv.tile([128, 512], f32, tag="pconv")
            mm_taps(b, t8, oc, ps, TAP_ORDER, True, True)
            return ps

        def conv_matmuls(b, t8):
            return [conv_matmuls_oc(b, t8, 0), conv_matmuls_oc(b, t8, 1)]

        def conv_evict_oc(b, t8, oc, ps, ob, last=False):
            d_sb = dsbs[b]
            o_sb = ospool.tile([128, 512], f16, tag="osb")
            # demod scale + fp32->fp16; oc0 on the ACT ring, oc1 on the
            # DVE so neither ring carries both evictions.  The final tile
            # evicts on the DVE only: the scalar ring may be mid-way through
            # a 1.2us xbar-transpose issue right when the tail starts.
            if oc == 0 and not last:
                nc.scalar.activation(o_sb, ps, AF.Copy, scale=d_sb[:, oc : oc + 1])
            else:
                nc.vector.tensor_scalar_mul(o_sb, ps, d_sb[:, oc : oc + 1])
            if last:
                # final tile: PE transpose (reusing the ingest psum pool,
                # idle by now) — shorter tail than xbar+DGE — and ship
                # each cout half as soon as it is ready
                pot_t = pxt.tile([128, 4, 128], f16, tag="pxt")
                for s in range(4):
                    nc.tensor.transpose(
                        pot_t[:, s, :], o_sb[:, s * 128 : (s + 1) * 128], ident_b
                    )
                nc.vector.tensor_copy(out=ob[:, :, oc * 128 : (oc + 1) * 128], in_=pot_t)
                yb = y_blk_ap(b, t8)
                half = bass.AP(
                    yb.tensor,
                    yb.offset + oc * 128,
                    [[COUT, 128], [128 * COUT, 4], [1, 128]],
                )
                nc.gpsimd.dma_start(out=half, in_=ob[:, :, oc * 128 : (oc + 1) * 128])
            else:
                # output transpose on the DMA xbar, split across rings
                eng = nc.sync if oc == 0 else nc.scalar
                eng.dma_start_transpose(out=ob[:, :, oc * 128 : (oc + 1) * 128], in_=o_sb)

        def conv_evict(b, t8, pss):
            ob = obpool.tile([128, 4, COUT], f16, tag="ob")
            for oc in range(2):
                conv_evict_oc(b, t8, oc, pss[oc], ob)
            nc.gpsimd.dma_start(out=y_blk_ap(b, t8), in_=ob)

        # K2[cin, cout] = sum_t kernel^2 (once per core).  Squared taps are
        # staged fp16 (2x DVE rate on the reduce read); accumulation and k2
        # stay fp32.  Emitted after the startup-critical DVE work; the demod
        # matmuls that consume it are deferred past conv tiles 0-1 so the PE
        # never waits for it.
        k2 = singles.tile([128, 2, COUT], f32)

        def compute_k2():
            for cc in range(2):
                k2tmp = tmp_pool.tile([128, T, COUT], f16, tag="k2tmp")
                nc.vector.tensor_mul(k2tmp, kbase[:, cc], kbase[:, cc])
                nc.vector.reduce_sum(
                    out=k2[:, cc],
                    in_=k2tmp.rearrange("p t c -> p c t"),
                    axis=mybir.AxisListType.X,
                )

        items = [(b, t8) for b in range(BPC) for t8 in range(8)]
        PF = 2  # transpose prefetch distance ahead of conv

        # Ramp: tiles 0 and 1 run with their taps split so transpose_block
        # k+1 is emitted between the parts — the PE does the taps that only
        # need blocks <=k while block k+1's DMA finishes, which both hides
        # the ingest latency AND guarantees >1.5us of slack between a
        # block's DMA completion and the PE transposes that read it (the
        # back-to-back case was a data race on the DMA's last descriptors).
        # Their evictions are deferred past demod so the demod matmuls
        # (which wait on K2) never sit in front of conv work in PE order.
        TAPS_LO = [1, 4, 0, 3, 2]  # touch nothing past pixel p0+511
        TAPS_HI = [5, 7, 6, 8]  # dx=+1 / dy=+1: read into the next block
        transpose_block(*items[0])
        ps0 = [pconv.tile([128, 512], f32, tag="pconv", name=f"ps0_{oc}") for oc in range(2)]
        for oc in range(2):
            mm_taps(0, 0, oc, ps0[oc], TAPS_LO, True, False)
        transpose_block(*items[1])
        for oc in range(2):
            mm_taps(0, 0, oc, ps0[oc], TAPS_HI, False, True)
        ps1 = [pconv.tile([128, 512], f32, tag="pconv", name=f"ps1_{oc}") for oc in range(2)]
        for oc in range(2):
            mm_taps(0, 1, oc, ps1[oc], TAPS_LO, True, False)
        transpose_block(*items[2])
        for oc in range(2):
            mm_taps(0, 1, oc, ps1[oc], TAPS_HI, False, True)
        transpose_block(*items[3])
        compute_k2()
        demod(0)
        if BPC > 1:
            demod(1)
        conv_evict(*items[0], ps0)
        conv_evict(*items[1], ps1)
        for i, (b, t8) in enumerate(items):
            if i < 2:
                continue
            if i + PF < len(items):
                transpose_block(*items[i + PF])
            if i == len(items) - 1:
                # last tile per-oc: oc0's evict/transpose/store overlaps
                # oc1's matmuls, shortening the tail
                ob = obpool.tile([128, 4, COUT], f16, tag="ob")
                for oc in range(2):
                    ps = conv_matmuls_oc(b, t8, oc)
                    conv_evict_oc(b, t8, oc, ps, ob, last=True)
            else:
                conv_evict(b, t8, conv_matmuls(b, t8))

    nc.compile()
    return nc


def _get_nc():
    if "nc" not in _CACHE:
        _CACHE["nc"] = _build_nc()
    return _CACHE["nc"]


def kernel(x, style, kernel, _trace=False):
    global LAST_EXEC_NS, LAST_MEAN_EXEC_NS
    from concourse.bass_utils import run_bass_kernel_spmd

    x = np.ascontiguousarray(x, dtype=np.float32)
    style = np.ascontiguousarray(style, dtype=np.float32)
    kern = np.ascontiguousarray(kernel, dtype=np.float32)

    nc = _get_nc()
    in_maps = [
        {
            "x": x[i * BPC : (i + 1) * BPC],
            "style": style[i * BPC : (i + 1) * BPC],
            "kernel": kern,
        }
        for i in range(NCORES)
    ]
    res = run_bass_kernel_spmd(nc, in_maps, core_ids=list(range(NCORES)), trace=_trace)
    LAST_EXEC_NS = res.exec_time_ns
    LAST_MEAN_EXEC_NS = res.mean_exec_time_ns
    return np.concatenate([res.results[i]["y"] for i in range(NCORES)], axis=0)
